# revision 16
# baseline (speedup 1.0000x reference)
"""Trainium2 Bass kernel for nn_DynamicPrototypeInteraction.

Data-parallel over B across 8 NeuronCores (2 batches/core); weights replicated.
See DESIGN.md (not required at runtime) for the derivation.
"""

import math

import numpy as np

import concourse.bass as bass
import concourse.mybir as mybir
import concourse.tile as tile
from concourse.bass_utils import run_bass_kernel_spmd
from concourse.masks import make_identity

F32 = mybir.dt.float32
BF16 = mybir.dt.bfloat16

EPS_NORM = 1e-12
BN_EPS = 1e-5
TEMP = 0.07

B, C, H, W = 16, 256, 64, 64
KD, P = 128, 256
HW = H * W
NCORES = 8
B_LOC = B // NCORES  # batches per core
N = 512  # strip size (8 H-rows)
NSTRIPS = HW // N
ROWS = N // W  # H-rows per strip
HP, WP = H + 2, W + 2  # padded spatial for depthwise

AF = mybir.ActivationFunctionType
OP = mybir.AluOpType

# taps handled by DVE vs GPSIMD (indices into the 9 (dy,dx) taps, row-major).
# tap 0 initializes the accumulator (must be first, runs on DVE).
DVE_TAPS = (0, 1, 2, 3, 4, 5, 6, 7, 8)
POOL_TAPS = ()


def _emit(nc, ob_zero, b2_zero):
    """Emit the full per-core program into nc (under TileContext)."""
    xh = nc.declare_dram_parameter("x", [B_LOC, C, H, W], F32, isOutput=False)
    mh = nc.declare_dram_parameter("m_features", [P, KD], F32, isOutput=False)
    qwh = nc.declare_dram_parameter("q_w", [KD, C], F32, isOutput=False)
    qbh = nc.declare_dram_parameter("q_b", [KD], F32, isOutput=False)
    kwh = nc.declare_dram_parameter("k_w", [KD, KD], F32, isOutput=False)
    kbh = nc.declare_dram_parameter("k_b", [KD], F32, isOutput=False)
    vwh = nc.declare_dram_parameter("v_w", [KD, KD], F32, isOutput=False)
    vbh = nc.declare_dram_parameter("v_b", [KD], F32, isOutput=False)
    owh = nc.declare_dram_parameter("o_w", [C, KD], F32, isOutput=False)
    obh = nc.declare_dram_parameter("o_b", [C], F32, isOutput=False)
    dwh = nc.declare_dram_parameter("dw_w", [C, 1, 3, 3], F32, isOutput=False)
    bn1wh = nc.declare_dram_parameter("bn1_w", [C], F32, isOutput=False)
    bn1bh = nc.declare_dram_parameter("bn1_b", [C], F32, isOutput=False)
    bn2wh = nc.declare_dram_parameter("bn2_w", [C], F32, isOutput=False)
    bn2bh = nc.declare_dram_parameter("bn2_b", [C], F32, isOutput=False)
    gamh = nc.declare_dram_parameter("gamma", [C], F32, isOutput=False)
    outh = nc.declare_dram_parameter("out", [B_LOC, C, H, W], F32, isOutput=True)

    x_v = xh[:].rearrange("b c h w -> b c (h w)")
    out_v = outh[:].rearrange("b c h w -> b c (h w)")
    scr_rn = nc.dram_tensor("scr_rn", [B_LOC, NSTRIPS, N], F32)
    scr_rd = nc.dram_tensor("scr_rd", [B_LOC, NSTRIPS, N], F32)
    scr_small = nc.dram_tensor("scr_small", [4, P], F32)

    def bcast_from_dram(dram_ap, nparts=128):
        """AP replicating a flat DRAM row across SBUF partitions."""
        flat = dram_ap
        return bass.AP(tensor=flat.tensor, offset=flat.offset,
                       ap=[[0, nparts]] + [list(d) for d in flat.ap])
    rsq = 1.0 / math.sqrt(1.0 + BN_EPS)  # bn eval scale factor

    from contextlib import ExitStack

    with tile.TileContext(nc) as tc, ExitStack() as stack:
        cst = stack.enter_context(tc.tile_pool(name="cst", bufs=1))
        setup_stack = ExitStack()
        tmp = setup_stack.enter_context(tc.tile_pool(name="tmp", bufs=1))
        st_ps = setup_stack.enter_context(tc.tile_pool(name="st_ps", bufs=1, space="PSUM"))

        # ---------------- constants / setup ----------------
        ones_col = cst.tile([128, 1], F32)
        nc.vector.memset(ones_col, 1.0)
        ones_col_bf = cst.tile([128, 1], BF16)
        nc.vector.memset(ones_col_bf, 1.0)
        ones_row = cst.tile([1, N], F32)
        nc.vector.memset(ones_row, 1.0)
        ident = cst.tile([128, 128], F32)
        make_identity(nc, ident)


        def load_T(dram_ap, dst):
            """dst[SBUF 128x128] = dram_ap.T via PE transpose (setup only)."""
            raw = tmp.tile([128, 128], F32, tag="rawT")
            nc.gpsimd.dma_start(out=raw, in_=dram_ap)
            tp = st_ps.tile([128, 128], F32, tag="tpT")
            nc.tensor.transpose(tp, raw, ident)
            nc.vector.tensor_copy(dst, tp)

        qb_sb = cst.tile([128, 1], F32)
        nc.gpsimd.dma_start(out=qb_sb, in_=qbh[:].rearrange("(k o) -> k o", o=1))
        kb_sb = cst.tile([128, 1], F32)
        nc.gpsimd.dma_start(out=kb_sb, in_=kbh[:].rearrange("(k o) -> k o", o=1))

        # per C-chunk [128,1] param columns
        s2_col, gs2_col, s1p_col, b1_col, b2_col, gob_col = [], [], [], [], [], []
        qwT, owT_g, diag_s2, w_taps, b2_row = [], [], [], [], []
        for ch in range(2):
            c0 = ch * 128
            sl = slice(c0, c0 + 128)

            t = cst.tile([128, 1], F32, tag=f"s2_{ch}")
            nc.gpsimd.dma_start(out=t, in_=bn2wh[sl].rearrange("(c o) -> c o", o=1))
            nc.vector.tensor_scalar_mul(out=t, in0=t, scalar1=rsq)  # s2, exact on DVE
            s2_col.append(t)

            g = cst.tile([128, 1], F32, tag=f"gam_{ch}")
            nc.gpsimd.dma_start(out=g, in_=gamh[sl].rearrange("(c o) -> c o", o=1))
            gs2 = cst.tile([128, 1], F32, tag=f"gs2_{ch}")
            nc.vector.tensor_mul(gs2, g, t)  # gamma*s2
            gs2_col.append(gs2)

            s1 = cst.tile([128, 1], F32, tag=f"s1_{ch}")
            nc.gpsimd.dma_start(out=s1, in_=bn1wh[sl].rearrange("(c o) -> c o", o=1))
            nc.scalar.mul(out=s1, in_=s1, mul=rsq)
            rgs2 = tmp.tile([128, 1], F32, tag="rgs2")
            nc.vector.reciprocal(rgs2, gs2)
            s1p = cst.tile([128, 1], F32, tag=f"s1p_{ch}")
            nc.vector.tensor_mul(s1p, s1, rgs2)  # bn1_s/(gamma*s2)
            s1p_col.append(s1p)

            b1 = cst.tile([128, 1], F32, tag=f"b1_{ch}")
            nc.gpsimd.dma_start(out=b1, in_=bn1bh[sl].rearrange("(c o) -> c o", o=1))
            b1_col.append(b1)

            b2 = cst.tile([1, 128], F32, tag=f"b2r_{ch}")
            nc.gpsimd.dma_start(out=b2, in_=bn2bh[sl].rearrange("(o c) -> o c", o=1))
            b2_row.append(b2)

            if not ob_zero:
                ob = cst.tile([128, 1], F32, tag=f"ob_{ch}")
                nc.gpsimd.dma_start(out=ob, in_=obh[sl].rearrange("(c o) -> c o", o=1))
                gob = cst.tile([128, 1], F32, tag=f"gob_{ch}")
                nc.vector.tensor_mul(gob, gs2, ob)  # (gamma*s2)*o_b
                gob_col.append(gob)

            qw = cst.tile([128, 128], F32, tag=f"qwT_{ch}")
            load_T(qwh[:, sl], qw)
            qwT.append(qw)

            ow = cst.tile([128, 128], F32, tag=f"owT_{ch}")
            load_T(owh[sl, :], ow)
            # scale columns (output channels) by gamma*s2: build row then bcast
            gs2r = tmp.tile([1, 128], F32, tag="gs2r")
            nc.gpsimd.dma_start(out=gs2r, in_=gamh[sl].rearrange("(o c) -> o c", o=1))
            bn2r = tmp.tile([1, 128], F32, tag="bn2r")
            nc.gpsimd.dma_start(out=bn2r, in_=bn2wh[sl].rearrange("(o c) -> o c", o=1))
            nc.scalar.mul(out=bn2r, in_=bn2r, mul=rsq)
            nc.vector.tensor_mul(gs2r, gs2r, bn2r)
            nc.gpsimd.dma_start(out=scr_small[ch, 0:128], in_=gs2r)
            gs2bc = tmp.tile([128, 128], F32, tag="gs2bc")
            nc.gpsimd.dma_start(out=gs2bc, in_=bcast_from_dram(scr_small[ch, 0:128]))
            nc.vector.tensor_mul(ow, ow, gs2bc)
            owT_g.append(ow)

            wt = cst.tile([128, 9], F32, tag=f"wt_{ch}")
            nc.gpsimd.dma_start(out=wt, in_=dwh[sl].rearrange("c o a b -> c (o a b)"))
            w_taps.append(wt)

        # ---- k / v from memory bank ----
        mT = cst.tile([128, P], F32)  # m^T [kd_in, p]
        load_T(mh[0:128, :], mT[:, 0:128])
        load_T(mh[128:256, :], mT[:, 128:256])
        kwT = tmp.tile([128, 128], F32, tag="kwT")  # [k_in, k_out]
        load_T(kwh[:], kwT)
        vwT = tmp.tile([128, 128], F32, tag="vwT")
        load_T(vwh[:], vwT)

        kpre = st_ps.tile([128, P], F32, tag="kpre")  # kT_pre [k_out, p]
        nc.tensor.matmul(kpre, kwT, mT, start=True, stop=True)
        k2 = tmp.tile([128, P], F32, tag="k2")
        nc.scalar.activation(out=k2, in_=kpre, func=AF.Square, bias=kb_sb, scale=1.0)
        ssqk = st_ps.tile([1, P], F32, tag="ssqk")
        nc.tensor.matmul(ssqk, ones_col, k2, start=True, stop=True)
        ssqk_row = tmp.tile([1, P], F32, tag="ssqk_row")
        nc.scalar.copy(out=ssqk_row, in_=ssqk)
        ssqk_t = tmp.tile([128, 2], F32, tag="ssqk_t")
        nc.gpsimd.dma_start(out=ssqk_t, in_=ssqk_row)
        nc.scalar.sqrt(out=ssqk_t, in_=ssqk_t)
        nc.vector.tensor_scalar_max(out=ssqk_t, in0=ssqk_t, scalar1=EPS_NORM)
        nc.vector.reciprocal(ssqk_t, ssqk_t)
        nc.scalar.mul(out=ssqk_t, in_=ssqk_t, mul=1.0 / TEMP)
        nc.gpsimd.dma_start(out=scr_small[2], in_=ssqk_t)
        rnk_bc = tmp.tile([128, P], F32, tag="rnk_bc")
        nc.gpsimd.dma_start(out=rnk_bc, in_=bcast_from_dram(scr_small[2]))
        kTs = cst.tile([128, P], F32)  # normalized k^T, pre-scaled 1/TEMP
        nc.vector.scalar_tensor_tensor(
            out=kTs, in0=kpre, scalar=kb_sb, in1=rnk_bc, op0=OP.add, op1=OP.mult
        )

        vb_bc = tmp.tile([128, 128], F32, tag="vb_bc")
        nc.gpsimd.dma_start(out=vb_bc, in_=bcast_from_dram(vbh[:]))
        v_bf = []
        for ch in range(2):
            vps = st_ps.tile([128, 128], F32, tag="vps")
            nc.tensor.matmul(
                vps, mT[:, ch * 128 : (ch + 1) * 128], vwT, start=True, stop=True
            )
            vb16 = cst.tile([128, 128], BF16, tag=f"v_{ch}")
            nc.vector.tensor_add(vb16, vps, vb_bc)
            v_bf.append(vb16)

        # ---------------- pools for the main pipeline ----------------
        setup_stack.close()
        xp = stack.enter_context(tc.tile_pool(name="xp", bufs=2))
        ypadp = stack.enter_context(tc.tile_pool(name="ypad", bufs=2))
        sb = stack.enter_context(tc.tile_pool(name="sb", bufs=2))
        att_p = stack.enter_context(tc.tile_pool(name="attp", bufs=2))
        tiny = stack.enter_context(tc.tile_pool(name="tiny", bufs=4))
        row_p = stack.enter_context(tc.tile_pool(name="rowp", bufs=2))
        bc_p = stack.enter_context(tc.tile_pool(name="bcp", bufs=2))
        res_p = stack.enter_context(tc.tile_pool(name="resp", bufs=2))
        lacc_p = stack.enter_context(tc.tile_pool(name="laccp", bufs=2))

        mm_ps = stack.enter_context(tc.tile_pool(name="mm_ps", bufs=2, space="PSUM"))
        lg_ps = stack.enter_context(tc.tile_pool(name="lg_ps", bufs=2, space="PSUM"))
        y_ps_pool = stack.enter_context(tc.tile_pool(name="y_ps", bufs=4, space="PSUM"))

        for b in range(B_LOC):
            x_sb = []
            for ch in range(2):
                xt = xp.tile([128, HW], F32, tag=f"x_{ch}")
                nc.sync.dma_start(out=xt, in_=x_v[b, ch * 128 : (ch + 1) * 128, :])
                x_sb.append(xt)

            y_pad = []
            for ch in range(2):
                yp = ypadp.tile([128, HP, WP], BF16, tag=f"yp_{ch}")
                # zero the borders (top/bottom rows, left/right cols)
                nc.vector.memset(yp[:, 0, :], 0.0)
                nc.vector.memset(yp[:, HP - 1, :], 0.0)
                nc.vector.memset(yp[:, 1 : HP - 1, 0:1], 0.0)
                nc.vector.memset(yp[:, 1 : HP - 1, WP - 1 : WP], 0.0)
                y_pad.append(yp)

            y_pre = [[None, None] for _ in range(NSTRIPS)]  # PSUM PRE tiles

            def do_dw_tail(s):
                """Depthwise conv + gelu + final combine for strip s (rows 8s..8s+8)."""
                r0 = s * ROWS
                for ch in range(2):
                    lacc = lacc_p.tile([128, ROWS, W], BF16, tag=f"lacc_{ch}")
                    first = True
                    for t in DVE_TAPS + POOL_TAPS:
                        dy, dx = divmod(t, 3)
                        src = y_pad[ch][:, r0 + dy : r0 + dy + ROWS, dx : dx + W]
                        wsc = w_taps[ch][:, t : t + 1]
                        if first:
                            nc.vector.tensor_scalar_mul(lacc, src, wsc)
                            first = False
                        else:
                            nc.vector.scalar_tensor_tensor(
                                out=lacc, in0=src, scalar=wsc, in1=lacc,
                                op0=OP.mult, op1=OP.add,
                            )
                    local2 = sb.tile([128, N], F32, tag=f"loc_{ch}")
                    nc.scalar.activation(
                        out=local2, in_=lacc.rearrange("p a b -> p (a b)"),
                        func=AF.Gelu, bias=b1_col[ch], scale=s1p_col[ch],
                    )
                    res_t = sb.tile([128, N], F32, tag=f"rest_{ch}")
                    nc.vector.scalar_tensor_tensor(
                        out=res_t, in0=local2, scalar=gs2_col[ch], in1=y_pre[s][ch],
                        op0=OP.mult, op1=OP.add,
                    )
                    res = res_p.tile([128, N], F32, tag=f"res_{ch}")
                    nc.vector.scalar_tensor_tensor(
                        out=res, in0=x_sb[ch][:, s * N : (s + 1) * N],
                        scalar=s2_col[ch], in1=res_t, op0=OP.mult, op1=OP.add,
                    )
                    nc.sync.dma_start(
                        out=out_v[b, ch * 128 : (ch + 1) * 128, s * N : (s + 1) * N],
                        in_=res,
                    )

            for s in range(NSTRIPS):
                n0 = s * N
                nsl = slice(n0, n0 + N)
                # 1. q projection
                q_psum = mm_ps.tile([128, N], F32, tag="mm")
                nc.tensor.matmul(q_psum, qwT[0], x_sb[0][:, nsl], start=True, stop=False)
                nc.tensor.matmul(q_psum, qwT[1], x_sb[1][:, nsl], start=False, stop=True)
                # 2-4. ssq -> rnorm broadcast
                q2 = sb.tile([128, N], F32, tag="q2")
                nc.scalar.activation(out=q2, in_=q_psum, func=AF.Square, bias=qb_sb, scale=1.0)
                ssq = mm_ps.tile([1, N], F32, tag="mm")
                nc.tensor.matmul(ssq, ones_col, q2, start=True, stop=True)
                ssq_row = row_p.tile([1, N], F32, tag="ssq_row")
                nc.scalar.copy(out=ssq_row, in_=ssq)
                ssq_t = tiny.tile([128, N // 128], F32, tag="ssq_t")
                nc.gpsimd.dma_start(out=ssq_t, in_=ssq_row)
                nc.scalar.sqrt(out=ssq_t, in_=ssq_t)
                nc.vector.tensor_scalar_max(out=ssq_t, in0=ssq_t, scalar1=EPS_NORM)
                nc.vector.reciprocal(ssq_t, ssq_t)
                nc.gpsimd.dma_start(out=scr_rn[b, s], in_=ssq_t)
                rn_bc = bc_p.tile([128, N], F32, tag="rn_bc")
                nc.gpsimd.dma_start(out=rn_bc, in_=bcast_from_dram(scr_rn[b, s]))
                # 5. normalized q
                q_n = sb.tile([128, N], F32, tag="q_n")
                nc.vector.scalar_tensor_tensor(
                    out=q_n, in0=q_psum, scalar=qb_sb, in1=rn_bc, op0=OP.add, op1=OP.mult
                )
                # 6-7. logits -> exp (bf16)
                att = []
                for ch in range(2):
                    lg = lg_ps.tile([128, N], F32, tag="lg")
                    nc.tensor.matmul(
                        lg, kTs[:, ch * 128 : (ch + 1) * 128], q_n, start=True, stop=True
                    )
                    a = att_p.tile([128, N], BF16, tag=f"att_{ch}")
                    nc.scalar.activation(out=a, in_=lg, func=AF.Exp)
                    att.append(a)
                # 8-9. denominator -> reciprocal broadcast
                den = mm_ps.tile([1, N], F32, tag="mm")
                nc.tensor.matmul(den, ones_col_bf, att[0], start=True, stop=False)
                nc.tensor.matmul(den, ones_col_bf, att[1], start=False, stop=True)
                den_row = row_p.tile([1, N], F32, tag="den_row")
                nc.vector.tensor_copy(den_row, den)
                den_t = tiny.tile([128, N // 128], F32, tag="den_t")
                nc.gpsimd.dma_start(out=den_t, in_=den_row)
                nc.vector.reciprocal(den_t, den_t)
                nc.gpsimd.dma_start(out=scr_rd[b, s], in_=den_t)
                rd_bc = bc_p.tile([128, N], F32, tag="rd_bc")
                nc.gpsimd.dma_start(out=rd_bc, in_=bcast_from_dram(scr_rd[b, s]))
                # 10-11. attn @ v, normalize
                ou = mm_ps.tile([128, N], F32, tag="mm")
                nc.tensor.matmul(ou, v_bf[0], att[0], start=True, stop=False)
                nc.tensor.matmul(ou, v_bf[1], att[1], start=False, stop=True)
                ou_n = sb.tile([128, N], F32, tag="ou_n")
                nc.vector.tensor_mul(ou_n, ou, rd_bc)
                # 12-14. o_proj (pre-scaled), pad-copy, PRE accumulation
                for ch in range(2):
                    yps = y_ps_pool.tile([128, N], F32, tag="y")
                    nc.tensor.matmul(yps, owT_g[ch], ou_n, start=True, stop=b2_zero)
                    # y'' -> padded bf16 tile (rows 8s+1 .. 8s+9, cols 1..65)
                    dst = y_pad[ch][:, s * ROWS + 1 : s * ROWS + 1 + ROWS, 1 : 1 + W]
                    if ob_zero:
                        nc.scalar.activation(out=dst, in_=yps, func=AF.Identity)
                    else:
                        nc.scalar.activation(
                            out=dst, in_=yps, func=AF.Identity, bias=gob_col[ch], scale=1.0
                        )
                    # PRE = y'' (+ b2 via K=1 matmul in the general path)
                    if not b2_zero:
                        nc.tensor.matmul(yps, b2_row[ch], ones_row, start=False, stop=True)
                    y_pre[s][ch] = yps
                if s > 0:
                    do_dw_tail(s - 1)
            do_dw_tail(NSTRIPS - 1)
    return nc


def _legalize_waits(nc):
    """This toolchain's walrus accepts at most ONE sync wait per instruction.

    Tile emits several on fan-in instructions; hoist the extras onto injected
    NoOps placed immediately before the instruction on the same engine (safe:
    identical execution point, and this kernel has no sem-resetting loops).
    """
    for f in nc.m.functions:
        for blk in f.blocks:
            il = blk.instructions
            i = 0
            ctr = 0
            while i < len(il):
                inst = il[i]
                si = getattr(inst, "sync_info", None)
                if si is not None and si.on_wait and len(si.on_wait) > 1:
                    waits = list(si.on_wait)
                    for w in waits[:-1]:
                        nop = mybir.InstNoOp(
                            name=f"{inst.name}_xw{ctr}",
                            engine=inst.engine,
                            ins=[],
                            outs=[],
                        )
                        nop.sync_info = mybir.SyncInfo(on_wait=[w], on_update=[])
                        il.insert(i, nop)
                        i += 1
                        ctr += 1
                    inst.sync_info = mybir.SyncInfo(
                        on_wait=[waits[-1]], on_update=list(si.on_update)
                    )
                i += 1
            blk.instructions = il


_CACHE = {}


def _build(ob_zero, b2_zero):
    key = (ob_zero, b2_zero)
    if key not in _CACHE:
        nc = bass.Bass()
        _emit(nc, ob_zero, b2_zero)
        _legalize_waits(nc)
        _CACHE[key] = nc
    return _CACHE[key]


def kernel(**inputs):
    x = np.ascontiguousarray(np.asarray(inputs["x"], dtype=np.float32))
    ob_zero = not np.any(np.asarray(inputs["o_b"]))
    b2_zero = not np.any(np.asarray(inputs["bn2_b"]))
    nc = _build(ob_zero, b2_zero)

    weights = {
        k: np.ascontiguousarray(np.asarray(inputs[k], dtype=np.float32))
        for k in (
            "m_features", "q_w", "q_b", "k_w", "k_b", "v_w", "v_b", "o_w", "o_b",
            "dw_w", "bn1_w", "bn1_b", "bn2_w", "bn2_b", "gamma",
        )
    }
    in_maps = []
    for core in range(NCORES):
        m = dict(weights)
        m["x"] = x[core * B_LOC : (core + 1) * B_LOC]
        in_maps.append(m)
    res = run_bass_kernel_spmd(nc, in_maps, list(range(NCORES)))
    return np.concatenate([r["out"] for r in res.results], axis=0)
